# revision 9
# baseline (speedup 1.0000x reference)
"""Trainium2 Bass kernel for nn_Attention_9844065042780.

Sharding: expert-parallel over the K=8 independent groups, one group per
NeuronCore (8 cores).  Each core receives the full activations x (reordered
host-side), the full conv_w (to compute the shared softmax attention maps
and the shared orthogonality loss), and only its own group's
dimred/Wo/Wg weights.  Outputs are disjoint per-core slices (hyp[:,k,:],
conf[:,k]) plus the (identical on every core) loss, gathered host-side.

Per-core math (k = this core's group):
  z[k', hw, b]   = sum_n conv_w[k', n] x[b, n, hw]            (PE)
  ah[k', hw, b]  = softmax_k'(z)                              (ACT exp, PE sum, DVE recip/mul)
  y[(hw,n), b]   = ah[k, hw, b] * x[b, n, hw]                 (PE row-broadcast + DVE mul)
  dim_red[b, m]  = sum_{hw,n} y * wd[m, n, hw] + db[m]        (PE, 417 accumulating matmuls)
  hyp[c, b]      = sum_n Wo[c, n] dim_red[b, n] + Wo_b[c]     (PE + ACT bias)
  conf[b]        = tanh(sum_n Wg[n] dim_red[b, n] + Wg_b)     (PE + ACT)
  loss[b]        = ||A_b^T A_b||_F^2 - sum((H^T A_b)^2),  A_b = ah[:, :, :, b]  (PE/ACT/DVE)

Layouts (host-prepared, hw padded 196->208 so slices are uniform 16*32=512):
  x_t  [2, 208, 128, 32]  bf16   x_t[nh, hw, nl, b] = x[b, nh*128+nl, hw]
  wd_t [2, 208, 128, 256] bf16   wd_t[nh, hw, nl, m] = dimred_w[k, m, nh*128+nl, hw]
"""

import os
import numpy as np
import ml_dtypes
from contextlib import ExitStack

from concourse import bass, bacc, tile, mybir
from concourse.bass_utils import run_bass_kernel_spmd

F32 = mybir.dt.float32
BF16 = mybir.dt.bfloat16
AF = mybir.ActivationFunctionType
ALU = mybir.AluOpType
AX = mybir.AxisListType

B, N, H, W, K, C = 32, 256, 14, 14, 8, 1000
HW, HWP, CP = 196, 196, 1024
SW = 14                      # hw positions per slice (one h row)
NSL = HWP // SW              # 14 slices
SLW = SW * B                 # 448 free columns per slice
NCORES = 8


def build_graph():
    nc = bacc.Bacc("TRN2", target_bir_lowering=False, debug=False)

    def inp(name, shape, dtype):
        return nc.dram_tensor(name, shape, dtype, kind="ExternalInput").ap()

    def outp(name, shape, dtype):
        return nc.dram_tensor(name, shape, dtype, kind="ExternalOutput").ap()

    x_d = inp("x_t", [2, HWP, 128, B], BF16)
    wd_d = inp("wd_t", [2, HWP, 128, N], BF16)
    cw_d = inp("cw_t", [2, 128, K], BF16)
    wo_d = inp("wo_t", [2, 128, CP], BF16)
    wob_d = inp("wob_t", [CP // 128, 128], F32)
    wg_d = inp("wg_t", [2, 128, 1], BF16)
    wgb_d = inp("wgb_t", [1, 1], F32)
    db_d = inp("db_t", [1, N], F32)
    ones8_d = inp("ones8", [8, 8], F32)
    selk_d = inp("selk", [8, 128], F32)
    hsel_d = inp("hsel", [K * H, K], F32)
    ones14_d = inp("ones14", [14, 1], F32)
    mones8_d = inp("mones8", [8, 1], F32)
    onesb_d = inp("onesb", [1, B], F32)
    id32_d = inp("id32", [32, 32], BF16)

    hyp_d = outp("out_hyp", [CP // 128, 128, B], F32)
    conf_d = outp("out_conf", [1, B], F32)
    loss_d = outp("out_loss", [1, B], F32)

    with tile.TileContext(nc) as tc, ExitStack() as ctx:
        const = ctx.enter_context(tc.tile_pool(name="const", bufs=1))
        persist = ctx.enter_context(tc.tile_pool(name="persist", bufs=1))
        dram = ctx.enter_context(tc.tile_pool(name="dram", bufs=1, space="DRAM"))
        dr_pool = ctx.enter_context(
            tc.tile_pool(name="dr_psum", bufs=1, space="PSUM")
        )

        # ---- constants into SBUF ----
        cw_sb = const.tile([128, 2 * K], BF16)
        nc.sync.dma_start(cw_sb.rearrange("p (t c) -> p t c", t=2),
                          cw_d.rearrange("t p c -> p t c"))
        wo_sb = const.tile([128, 2 * CP], BF16)
        nc.sync.dma_start(wo_sb.rearrange("p (t c) -> p t c", t=2),
                          wo_d.rearrange("t p c -> p t c"))
        wob_sb = const.tile([128, CP // 128], F32)
        nc.sync.dma_start(wob_sb, wob_d.rearrange("c p -> p c"))
        wg_sb = const.tile([128, 2], BF16)
        nc.sync.dma_start(wg_sb.rearrange("p (t one) -> p t one", t=2),
                          wg_d.rearrange("t p one -> p t one"))
        wgb_sb = const.tile([1, 1], F32)
        nc.sync.dma_start(wgb_sb, wgb_d)
        db_sb = const.tile([1, N], F32)
        nc.sync.dma_start(db_sb, db_d)
        ones8_sb = const.tile([8, 8], F32)
        nc.sync.dma_start(ones8_sb, ones8_d)
        selk_sb = const.tile([8, 128], F32)
        nc.sync.dma_start(selk_sb, selk_d)
        hsel_sb = const.tile([K * H, K], F32)
        nc.sync.dma_start(hsel_sb, hsel_d)
        ones14_sb = const.tile([14, 1], F32)
        nc.sync.dma_start(ones14_sb, ones14_d)
        mones8_sb = const.tile([8, 1], F32)
        nc.sync.dma_start(mones8_sb, mones8_d)
        onesb_sb = const.tile([1, B], F32)
        nc.sync.dma_start(onesb_sb, onesb_d)
        id32_sb = const.tile([32, 32], BF16)
        nc.sync.dma_start(id32_sb, id32_d)

        # full attention map, all K groups: [8, (hw, b)] fp32
        ah_full = persist.tile([K, HWP * B], F32)

        # dim_red accumulator [b, m]
        dimred_ps = dr_pool.tile([B, N], F32, space="PSUM")
        # bias seeding matmul: dimred[b, m] = 1 * db[m]
        nc.tensor.matmul(
            dimred_ps, lhsT=onesb_sb, rhs=db_sb, start=True, stop=False,
            skip_group_check=True,
        )

        with ExitStack() as mctx:
            xp = mctx.enter_context(tc.tile_pool(name="xp", bufs=6))
            wdp = mctx.enter_context(tc.tile_pool(name="wdp", bufs=6))
            ep = mctx.enter_context(tc.tile_pool(name="ep", bufs=3))
            rp = mctx.enter_context(tc.tile_pool(name="rp", bufs=3))
            yp = mctx.enter_context(tc.tile_pool(name="yp", bufs=4))
            zp = mctx.enter_context(tc.tile_pool(name="zp", bufs=2, space="PSUM"))
            sp = mctx.enter_context(tc.tile_pool(name="sp", bufs=2, space="PSUM"))
            abp = mctx.enter_context(tc.tile_pool(name="abp", bufs=2, space="PSUM"))

            for s in range(NSL):
                hw0 = s * SW
                x_sb = xp.tile([128, 2 * SLW], BF16, tag="x")
                for u in range(2):
                    nc.sync.dma_start(
                        x_sb[:, u * SLW:(u + 1) * SLW].rearrange(
                            "p (t b) -> p t b", t=SW),
                        x_d[u, hw0:hw0 + SW].rearrange("t p b -> p t b"),
                    )
                # logits z[k', (hw b)]
                z_ps = zp.tile([K, SLW], F32, tag="z", space="PSUM")
                nc.tensor.matmul(
                    z_ps, lhsT=cw_sb[:, 0:K], rhs=x_sb[:, 0:SLW],
                    start=True, stop=False,
                )
                nc.tensor.matmul(
                    z_ps, lhsT=cw_sb[:, K:2 * K], rhs=x_sb[:, SLW:2 * SLW],
                    start=False, stop=True,
                )
                e_sb = ep.tile([K, SLW], F32, tag="e")
                nc.scalar.activation(e_sb, z_ps, AF.Exp)
                ssum_ps = sp.tile([K, SLW], F32, tag="ss", space="PSUM")
                nc.tensor.matmul(ssum_ps, lhsT=ones8_sb, rhs=e_sb,
                                 start=True, stop=True)
                r_sb = rp.tile([K, SLW], F32, tag="r")
                nc.vector.reciprocal(r_sb, ssum_ps)
                ahw = ah_full[:, s * SLW:(s + 1) * SLW]
                nc.vector.tensor_tensor(out=ahw, in0=e_sb, in1=r_sb, op=ALU.mult)
                # broadcast row k to 128 partitions
                ahb_ps = abp.tile([128, SLW], F32, tag="ab", space="PSUM")
                nc.tensor.matmul(ahb_ps, lhsT=selk_sb, rhs=ahw,
                                 start=True, stop=True)
                for nh in range(2):
                    y_sb = yp.tile([128, SLW], BF16, tag="y")
                    nc.vector.tensor_tensor(
                        out=y_sb, in0=x_sb[:, nh * SLW:(nh + 1) * SLW],
                        in1=ahb_ps, op=ALU.mult,
                    )
                    wd_sb = wdp.tile([128, SW * N], BF16, tag="wd")
                    nc.sync.dma_start(
                        wd_sb.rearrange("p (t m) -> p t m", t=SW),
                        wd_d[nh, hw0:hw0 + SW].rearrange("t p m -> p t m"),
                    )
                    last_slice = s == NSL - 1
                    for j in range(SW):
                        nc.tensor.matmul(
                            dimred_ps,
                            lhsT=y_sb[:, j * B:(j + 1) * B],
                            rhs=wd_sb[:, j * N:(j + 1) * N],
                            start=False,
                            stop=(last_slice and nh == 1 and j == SW - 1),
                            skip_group_check=True,
                        )

        # ---------------- tail ----------------
        with ExitStack() as tctx:
            tp = tctx.enter_context(tc.tile_pool(name="tail_sb", bufs=1))
            tpp = tctx.enter_context(
                tc.tile_pool(name="tail_ps", bufs=1, space="PSUM")
            )
            hp = tctx.enter_context(
                tc.tile_pool(name="hyp_ps", bufs=2, space="PSUM")
            )
            hs = tctx.enter_context(tc.tile_pool(name="hyp_sb", bufs=2))

            # dim_red -> SBUF bf16, then transpose to [n, b]
            dr_sb = tp.tile([B, N], BF16)
            nc.vector.tensor_copy(dr_sb, dimred_ps)
            drT_sb = tp.tile([128, 2 * B], BF16)
            for nh in range(2):
                drT_ps = tpp.tile([128, B], BF16, tag="drT", space="PSUM")
                nc.tensor.transpose(
                    drT_ps, dr_sb[:, nh * 128:(nh + 1) * 128], id32_sb
                )
                nc.vector.tensor_copy(drT_sb[:, nh * B:(nh + 1) * B], drT_ps)

            # hyp
            for c in range(CP // 128):
                hyp_ps = hp.tile([128, B], F32, tag="hyp", space="PSUM")
                nc.tensor.matmul(
                    hyp_ps, lhsT=wo_sb[:, c * 128:(c + 1) * 128],
                    rhs=drT_sb[:, 0:B], start=True, stop=False,
                )
                nc.tensor.matmul(
                    hyp_ps, lhsT=wo_sb[:, CP + c * 128:CP + (c + 1) * 128],
                    rhs=drT_sb[:, B:2 * B], start=False, stop=True,
                )
                hyp_sb = hs.tile([128, B], F32, tag="hyps")
                nc.scalar.activation(hyp_sb, hyp_ps, AF.Identity,
                                     bias=wob_sb[:, c:c + 1])
                nc.sync.dma_start(hyp_d[c], hyp_sb)

            # conf
            conf_ps = tpp.tile([1, B], F32, tag="conf", space="PSUM")
            nc.tensor.matmul(conf_ps, lhsT=wg_sb[:, 0:1], rhs=drT_sb[:, 0:B],
                             start=True, stop=False)
            nc.tensor.matmul(conf_ps, lhsT=wg_sb[:, 1:2], rhs=drT_sb[:, B:2 * B],
                             start=False, stop=True)
            conf_sb = tp.tile([1, B], F32)
            nc.scalar.activation(conf_sb, conf_ps, AF.Tanh, bias=wgb_sb[:, 0:1])
            nc.sync.dma_start(conf_d, conf_sb)

            # ---- loss ----
            # regather ah (hw < 196 region) to [(k h), (w b)] via DRAM bounce
            ah_dram = dram.tile([K, H * W * B], F32, space="DRAM")
            nc.sync.dma_start(ah_dram, ah_full[:, 0:H * W * B])
            A2 = tp.tile([K * H, W * B], F32)
            nc.sync.dma_start(
                A2, ah_dram.rearrange("k (h rest) -> (k h) rest", h=H)
            )
            A2v = A2.rearrange("p (w b) -> p w b", b=B)
            G_ps = tpp.tile([W, B * W], F32, tag="G", space="PSUM")
            for b in range(B):
                ab = A2v[:, :, b:b + 1]
                nc.tensor.matmul(G_ps[:, b * W:(b + 1) * W], lhsT=ab, rhs=ab,
                                 start=True, stop=True, skip_group_check=True)
            S_ps = tpp.tile([K, W * B], F32, tag="S", space="PSUM")
            nc.tensor.matmul(S_ps, lhsT=hsel_sb, rhs=A2, start=True, stop=True)
            Gsq = tp.tile([W, B * W], F32)
            nc.scalar.activation(Gsq, G_ps, AF.Square)
            Ssq = tp.tile([K, W * B], F32)
            nc.scalar.activation(Ssq, S_ps, AF.Square)
            Gred = tp.tile([W, B], F32)
            nc.vector.tensor_reduce(
                Gred, Gsq.rearrange("p (b v) -> p b v", b=B),
                axis=AX.X, op=ALU.add,
            )
            Sred = tp.tile([K, B], F32)
            nc.vector.tensor_reduce(
                Sred, Ssq.rearrange("p (w b) -> p b w", b=B),
                axis=AX.X, op=ALU.add,
            )
            l_ps = tpp.tile([1, B], F32, tag="l", space="PSUM")
            nc.tensor.matmul(l_ps, lhsT=ones14_sb, rhs=Gred,
                             start=True, stop=False)
            nc.tensor.matmul(l_ps, lhsT=mones8_sb, rhs=Sred,
                             start=False, stop=True)
            loss_sb = tp.tile([1, B], F32)
            nc.vector.tensor_copy(loss_sb, l_ps)
            nc.sync.dma_start(loss_d, loss_sb)

    return nc


def _bf16(a):
    return np.ascontiguousarray(a.astype(ml_dtypes.bfloat16))


def build_host_inputs(x, conv_w, dimred_w, dimred_b, Wo_w, Wo_b, Wg_w, Wg_b):
    """Returns in_maps: one dict per core."""
    x = np.asarray(x, np.float32)
    conv_w = np.asarray(conv_w, np.float32)
    dimred_w = np.asarray(dimred_w, np.float32)
    dimred_b = np.asarray(dimred_b, np.float32)
    Wo_w = np.asarray(Wo_w, np.float32)
    Wo_b = np.asarray(Wo_b, np.float32)
    Wg_w = np.asarray(Wg_w, np.float32)
    Wg_b = np.asarray(Wg_b, np.float32)

    # x_t[nh, hw, nl, b] = x[b, nh*128+nl, hw]
    xt = x.transpose(1, 2, 3, 0).reshape(N, HW, B)          # [n, hw, b]
    xt = xt.reshape(2, 128, HWP, B).transpose(0, 2, 1, 3)   # [2, hwp, 128, b]
    xt = _bf16(xt)

    # conv_w^T [2, 128, K]
    cwt = _bf16(conv_w.T.reshape(2, 128, K))

    shared = {
        "x_t": xt,
        "cw_t": cwt,
        "ones8": np.ones((8, 8), np.float32),
        "hsel": np.repeat(np.eye(K, dtype=np.float32), H, axis=0),
        "ones14": np.ones((14, 1), np.float32),
        "mones8": -np.ones((8, 1), np.float32),
        "onesb": np.ones((1, B), np.float32),
        "id32": _bf16(np.eye(32, dtype=np.float32)),
    }

    in_maps = []
    for k in range(NCORES):
        # wd_t[nh, hw, nl, m] = dimred_w[k, m, n, hw]
        wd = dimred_w[k].reshape(N, N, HW).transpose(1, 2, 0)   # [n, hw, m]
        wd = wd.reshape(2, 128, HWP, N).transpose(0, 2, 1, 3)
        wo = np.zeros((CP, N), np.float32)
        wo[:C] = Wo_w[k]
        wob = np.zeros((CP,), np.float32)
        wob[:C] = Wo_b[k]
        selk = np.zeros((8, 128), np.float32)
        selk[k] = 1.0
        m = dict(shared)
        m.update({
            "wd_t": _bf16(wd),
            "wo_t": _bf16(wo.T.reshape(2, 128, CP)),
            "wob_t": wob.reshape(CP // 128, 128),
            "wg_t": _bf16(Wg_w[k].reshape(2, 128, 1)),
            "wgb_t": np.full((1, 1), Wg_b[k], np.float32),
            "db_t": dimred_b[k].reshape(1, N),
            "selk": selk,
        })
        in_maps.append(m)
    return in_maps


def assemble_outputs(results):
    hyp = np.stack(
        [r["out_hyp"].reshape(CP, B)[:C].T for r in results], axis=1
    )                                                   # [B, K, C]
    conf = np.stack([r["out_conf"][0] for r in results], axis=1)[..., None]
    loss = results[0]["out_loss"][0][:, None]           # [B, 1]
    return (
        np.ascontiguousarray(hyp, np.float32),
        np.ascontiguousarray(conf, np.float32),
        np.ascontiguousarray(loss, np.float32),
    )


_GRAPH_CACHE = {}


def get_graph():
    if "nc" not in _GRAPH_CACHE:
        nc = build_graph()
        nc.finalize()
        _GRAPH_CACHE["nc"] = nc
    return _GRAPH_CACHE["nc"]


def kernel(**inputs):
    nc = get_graph()
    in_maps = build_host_inputs(**inputs)
    res = run_bass_kernel_spmd(nc, in_maps, core_ids=list(range(NCORES)))
    return assemble_outputs(res.results)


# revision 16
# speedup vs baseline: 1.3883x; 1.3883x over previous
"""Trainium2 Bass kernel for nn_Attention_9844065042780.

Sharding: expert-parallel over the K=8 independent groups, one group per
NeuronCore (8 cores).  Each core receives the full activations x (reordered
host-side), the full conv_w (to compute the shared softmax attention maps
and the shared orthogonality loss), and only its own group's
dimred/Wo/Wg weights.  Outputs are disjoint per-core slices (hyp[:,k,:],
conf[:,k]) plus the (identical on every core) loss, gathered host-side.

Per-core math (k = this core's group):
  z[k', hw, b]   = sum_n conv_w[k', n] x[b, n, hw]            (PE)
  ah[k', hw, b]  = softmax_k'(z)                              (ACT exp, PE sum, DVE recip/mul)
  y[(hw,n), b]   = ah[k, hw, b] * x[b, n, hw]                 (PE row-broadcast + DVE mul)
  dim_red[b, m]  = sum_{hw,n} y * wd[m, n, hw] + db[m]        (PE, 417 accumulating matmuls)
  hyp[c, b]      = sum_n Wo[c, n] dim_red[b, n] + Wo_b[c]     (PE + ACT bias)
  conf[b]        = tanh(sum_n Wg[n] dim_red[b, n] + Wg_b)     (PE + ACT)
  loss[b]        = ||A_b^T A_b||_F^2 - sum((H^T A_b)^2),  A_b = ah[:, :, :, b]  (PE/ACT/DVE)

Layouts (host-prepared, hw padded 196->208 so slices are uniform 16*32=512):
  x_t  [2, 208, 128, 32]  bf16   x_t[nh, hw, nl, b] = x[b, nh*128+nl, hw]
  wd_t [2, 208, 128, 256] bf16   wd_t[nh, hw, nl, m] = dimred_w[k, m, nh*128+nl, hw]
"""

import os
import numpy as np
import ml_dtypes
from contextlib import ExitStack

from concourse import bass, bacc, tile, mybir
from concourse.bass_utils import run_bass_kernel_spmd

F32 = mybir.dt.float32
BF16 = mybir.dt.bfloat16
AF = mybir.ActivationFunctionType
ALU = mybir.AluOpType
AX = mybir.AxisListType

B, N, H, W, K, C = 32, 256, 14, 14, 8, 1000
HW, HWP, CP = 196, 196, 1024
SW = 14                      # hw positions per slice (one h row)
NSL = HWP // SW              # 14 slices
SLW = SW * B                 # 448 free columns per slice
NCORES = 8


def build_graph():
    nc = bacc.Bacc("TRN2", target_bir_lowering=False, debug=False)

    def inp(name, shape, dtype):
        return nc.dram_tensor(name, shape, dtype, kind="ExternalInput").ap()

    def outp(name, shape, dtype):
        return nc.dram_tensor(name, shape, dtype, kind="ExternalOutput").ap()

    x_d = inp("x_t", [2, 128, HWP, B], BF16)
    wd_d = inp("wd_t", [2, 128, HWP, N], BF16)
    cw_d = inp("cw_t", [2, 128, K], BF16)
    wo_d = inp("wo_t", [2, 128, CP], BF16)
    wob_d = inp("wob_t", [CP // 128, 128], F32)
    wg_d = inp("wg_t", [2, 128, 1], BF16)
    wgb_d = inp("wgb_t", [1, 1], F32)
    db_d = inp("db_t", [1, N], F32)
    ones8_d = inp("ones8", [8, 8], F32)
    selk_d = inp("selk", [8, 128], F32)
    hsel_d = inp("hsel", [K * H, K], F32)
    ones14_d = inp("ones14", [14, 1], F32)
    mones8_d = inp("mones8", [8, 1], F32)
    onesb_d = inp("onesb", [1, B], F32)
    id32_d = inp("id32", [32, 32], BF16)

    hyp_d = outp("out_hyp", [CP // 128, 128, B], F32)
    conf_d = outp("out_conf", [1, B], F32)
    loss_d = outp("out_loss", [1, B], F32)

    with tile.TileContext(nc) as tc, ExitStack() as ctx:
        const = ctx.enter_context(tc.tile_pool(name="const", bufs=1))
        persist = ctx.enter_context(tc.tile_pool(name="persist", bufs=1))
        dram = ctx.enter_context(tc.tile_pool(name="dram", bufs=1, space="DRAM"))
        dr_pool = ctx.enter_context(
            tc.tile_pool(name="dr_psum", bufs=1, space="PSUM")
        )

        # ---- constants into SBUF ----
        cw_sb = const.tile([128, 2 * K], BF16)
        nc.sync.dma_start(cw_sb.rearrange("p (t c) -> p t c", t=2),
                          cw_d.rearrange("t p c -> p t c"))
        wo_sb = const.tile([128, 2 * CP], BF16)
        nc.sync.dma_start(wo_sb.rearrange("p (t c) -> p t c", t=2),
                          wo_d.rearrange("t p c -> p t c"))
        wob_sb = const.tile([128, CP // 128], F32)
        nc.sync.dma_start(wob_sb, wob_d.rearrange("c p -> p c"))
        wg_sb = const.tile([128, 2], BF16)
        nc.sync.dma_start(wg_sb.rearrange("p (t one) -> p t one", t=2),
                          wg_d.rearrange("t p one -> p t one"))
        wgb_sb = const.tile([1, 1], F32)
        nc.sync.dma_start(wgb_sb, wgb_d)
        db_sb = const.tile([1, N], F32)
        nc.sync.dma_start(db_sb, db_d)
        ones8_sb = const.tile([8, 8], F32)
        nc.sync.dma_start(ones8_sb, ones8_d)
        selk_sb = const.tile([8, 128], F32)
        nc.sync.dma_start(selk_sb, selk_d)
        hsel_sb = const.tile([K * H, K], F32)
        nc.sync.dma_start(hsel_sb, hsel_d)
        ones14_sb = const.tile([14, 1], F32)
        nc.sync.dma_start(ones14_sb, ones14_d)
        mones8_sb = const.tile([8, 1], F32)
        nc.sync.dma_start(mones8_sb, mones8_d)
        onesb_sb = const.tile([1, B], F32)
        nc.sync.dma_start(onesb_sb, onesb_d)
        id32_sb = const.tile([32, 32], BF16)
        nc.sync.dma_start(id32_sb, id32_d)

        # full attention map, all K groups: [8, (hw, b)] fp32
        ah_full = persist.tile([K, HWP * B], F32)

        # x resident in SBUF: [128, (u, hw, b)], loaded in 2 line-rate DMAs
        x_all = persist.tile([128, 2 * HWP * B], BF16)
        for u in range(2):
            nc.sync.dma_start(
                x_all[:, u * HWP * B:(u + 1) * HWP * B], x_d[u])

        # dim_red accumulator [b, m]
        dimred_ps = dr_pool.tile([B, N], F32, space="PSUM")
        # bias seeding matmul: dimred[b, m] = 1 * db[m]
        nc.tensor.matmul(
            dimred_ps, lhsT=onesb_sb, rhs=db_sb, start=True, stop=False,
            skip_group_check=True,
        )

        F32R = mybir.dt.float32r
        with ExitStack() as mctx:
            wdp = mctx.enter_context(tc.tile_pool(name="wdp", bufs=6))
            ep = mctx.enter_context(tc.tile_pool(name="ep", bufs=3))
            rp = mctx.enter_context(tc.tile_pool(name="rp", bufs=3))
            yp = mctx.enter_context(tc.tile_pool(name="yp", bufs=4))
            zp = mctx.enter_context(tc.tile_pool(name="zp", bufs=2, space="PSUM"))
            sp = mctx.enter_context(tc.tile_pool(name="sp", bufs=2, space="PSUM"))
            abp = mctx.enter_context(tc.tile_pool(name="abp", bufs=2, space="PSUM"))

            for s in range(NSL):
                hw0 = s * SW

                def xw(u):
                    return x_all[:, u * HWP * B + hw0 * B:
                                 u * HWP * B + (hw0 + SW) * B]

                # logits z[k', (hw b)]
                z_ps = zp.tile([K, SLW], F32, tag="z", space="PSUM")
                nc.tensor.matmul(
                    z_ps, lhsT=cw_sb[:, 0:K], rhs=xw(0),
                    start=True, stop=False,
                )
                nc.tensor.matmul(
                    z_ps, lhsT=cw_sb[:, K:2 * K], rhs=xw(1),
                    start=False, stop=True,
                )
                e_sb = ep.tile([K, SLW], F32, tag="e")
                nc.scalar.activation(e_sb, z_ps, AF.Exp)
                ssum_ps = sp.tile([K, SLW], F32, tag="ss", space="PSUM")
                nc.tensor.matmul(ssum_ps, lhsT=ones8_sb, rhs=e_sb,
                                 start=True, stop=True)
                r_sb = rp.tile([K, SLW], F32, tag="r")
                nc.vector.reciprocal_approx_fast(r_sb, ssum_ps)
                ahw = ah_full[:, s * SLW:(s + 1) * SLW]
                nc.vector.tensor_tensor(out=ahw, in0=e_sb, in1=r_sb, op=ALU.mult)
                # broadcast row k to 128 partitions
                ahb_ps = abp.tile([128, SLW], F32, tag="ab", space="PSUM")
                nc.tensor.matmul(ahb_ps, lhsT=selk_sb, rhs=ahw,
                                 start=True, stop=True)
                for nh in range(2):
                    y_sb = yp.tile([128, SLW], BF16, tag="y")
                    nc.vector.tensor_tensor(
                        out=y_sb, in0=xw(nh), in1=ahb_ps, op=ALU.mult,
                    )
                    wd_sb = wdp.tile([128, SW * N], BF16, tag="wd")
                    nc.sync.dma_start(wd_sb, wd_d[nh, :, hw0:hw0 + SW, :])
                    last_slice = s == NSL - 1
                    for j in range(SW):
                        nc.tensor.matmul(
                            dimred_ps,
                            lhsT=y_sb[:, j * B:(j + 1) * B],
                            rhs=wd_sb[:, j * N:(j + 1) * N],
                            start=False,
                            stop=(last_slice and nh == 1 and j == SW - 1),
                            skip_group_check=True,
                        )

        # ---------------- tail ----------------
        with ExitStack() as tctx:
            tp = tctx.enter_context(tc.tile_pool(name="tail_sb", bufs=1))
            tpp = tctx.enter_context(
                tc.tile_pool(name="tail_ps", bufs=1, space="PSUM")
            )
            hp = tctx.enter_context(
                tc.tile_pool(name="hyp_ps", bufs=2, space="PSUM")
            )
            hs = tctx.enter_context(tc.tile_pool(name="hyp_sb", bufs=2))

            # dim_red -> SBUF bf16, then transpose to [n, b]
            dr_sb = tp.tile([B, N], BF16)
            nc.vector.tensor_copy(dr_sb, dimred_ps)
            drT_sb = tp.tile([128, 2 * B], BF16)
            for nh in range(2):
                drT_ps = tpp.tile([128, B], BF16, tag="drT", space="PSUM")
                nc.tensor.transpose(
                    drT_ps, dr_sb[:, nh * 128:(nh + 1) * 128], id32_sb
                )
                nc.vector.tensor_copy(drT_sb[:, nh * B:(nh + 1) * B], drT_ps)

            # hyp
            for c in range(CP // 128):
                hyp_ps = hp.tile([128, B], F32, tag="hyp", space="PSUM")
                nc.tensor.matmul(
                    hyp_ps, lhsT=wo_sb[:, c * 128:(c + 1) * 128],
                    rhs=drT_sb[:, 0:B], start=True, stop=False,
                )
                nc.tensor.matmul(
                    hyp_ps, lhsT=wo_sb[:, CP + c * 128:CP + (c + 1) * 128],
                    rhs=drT_sb[:, B:2 * B], start=False, stop=True,
                )
                hyp_sb = hs.tile([128, B], F32, tag="hyps")
                nc.scalar.activation(hyp_sb, hyp_ps, AF.Identity,
                                     bias=wob_sb[:, c:c + 1])
                nc.sync.dma_start(hyp_d[c], hyp_sb)

            # conf
            conf_ps = tpp.tile([1, B], F32, tag="conf", space="PSUM")
            nc.tensor.matmul(conf_ps, lhsT=wg_sb[:, 0:1], rhs=drT_sb[:, 0:B],
                             start=True, stop=False)
            nc.tensor.matmul(conf_ps, lhsT=wg_sb[:, 1:2], rhs=drT_sb[:, B:2 * B],
                             start=False, stop=True)
            conf_sb = tp.tile([1, B], F32)
            nc.scalar.activation(conf_sb, conf_ps, AF.Tanh, bias=wgb_sb[:, 0:1])
            nc.sync.dma_start(conf_d, conf_sb)

            # ---- loss ----
            # regather ah (hw < 196 region) to [(k h), (w b)] via DRAM bounce
            ah_dram = dram.tile([K, H * W * B], F32, space="DRAM")
            nc.sync.dma_start(ah_dram, ah_full[:, 0:H * W * B])
            A2 = tp.tile([K * H, W * B], F32)
            nc.sync.dma_start(
                A2, ah_dram.rearrange("k (h rest) -> (k h) rest", h=H)
            )
            A2v = A2.rearrange("p (w b) -> p w b", b=B)
            G_ps = tpp.tile([W, B * W], F32, tag="G", space="PSUM")
            for b in range(B):
                ab = A2v[:, :, b:b + 1]
                nc.tensor.matmul(G_ps[:, b * W:(b + 1) * W], lhsT=ab, rhs=ab,
                                 start=True, stop=True, skip_group_check=True)
            S_ps = tpp.tile([K, W * B], F32, tag="S", space="PSUM")
            nc.tensor.matmul(S_ps, lhsT=hsel_sb, rhs=A2, start=True, stop=True)
            Gsq = tp.tile([W, B * W], F32)
            nc.scalar.activation(Gsq, G_ps, AF.Square)
            Ssq = tp.tile([K, W * B], F32)
            nc.scalar.activation(Ssq, S_ps, AF.Square)
            Gred = tp.tile([W, B], F32)
            nc.vector.tensor_reduce(
                Gred, Gsq.rearrange("p (b v) -> p b v", b=B),
                axis=AX.X, op=ALU.add,
            )
            Sred = tp.tile([K, B], F32)
            nc.vector.tensor_reduce(
                Sred, Ssq.rearrange("p (w b) -> p b w", b=B),
                axis=AX.X, op=ALU.add,
            )
            l_ps = tpp.tile([1, B], F32, tag="l", space="PSUM")
            nc.tensor.matmul(l_ps, lhsT=ones14_sb, rhs=Gred,
                             start=True, stop=False)
            nc.tensor.matmul(l_ps, lhsT=mones8_sb, rhs=Sred,
                             start=False, stop=True)
            loss_sb = tp.tile([1, B], F32)
            nc.vector.tensor_copy(loss_sb, l_ps)
            nc.sync.dma_start(loss_d, loss_sb)

    return nc


def _bf16(a):
    return np.ascontiguousarray(a.astype(ml_dtypes.bfloat16))


def build_host_inputs(x, conv_w, dimred_w, dimred_b, Wo_w, Wo_b, Wg_w, Wg_b):
    """Returns in_maps: one dict per core."""
    x = np.asarray(x, np.float32)
    conv_w = np.asarray(conv_w, np.float32)
    dimred_w = np.asarray(dimred_w, np.float32)
    dimred_b = np.asarray(dimred_b, np.float32)
    Wo_w = np.asarray(Wo_w, np.float32)
    Wo_b = np.asarray(Wo_b, np.float32)
    Wg_w = np.asarray(Wg_w, np.float32)
    Wg_b = np.asarray(Wg_b, np.float32)

    # x_t[nh, nl, hw, b] = x[b, nh*128+nl, hw]  (partition-major contiguous)
    xt = x.transpose(1, 2, 3, 0).reshape(N, HW, B)          # [n, hw, b]
    xt = _bf16(xt.reshape(2, 128, HWP, B))

    # conv_w^T [2, 128, K]
    cwt = _bf16(conv_w.T.reshape(2, 128, K))

    shared = {
        "x_t": xt,
        "cw_t": cwt,
        "ones8": np.ones((8, 8), np.float32),
        "hsel": np.repeat(np.eye(K, dtype=np.float32), H, axis=0),
        "ones14": np.ones((14, 1), np.float32),
        "mones8": -np.ones((8, 1), np.float32),
        "onesb": np.ones((1, B), np.float32),
        "id32": _bf16(np.eye(32, dtype=np.float32)),
    }

    in_maps = []
    for k in range(NCORES):
        # wd_t[nh, nl, hw, m] = dimred_w[k, m, n, hw]  (partition-major contiguous)
        wd = dimred_w[k].reshape(N, N, HW).transpose(1, 2, 0)   # [n, hw, m]
        wd = wd.reshape(2, 128, HWP, N)
        wo = np.zeros((CP, N), np.float32)
        wo[:C] = Wo_w[k]
        wob = np.zeros((CP,), np.float32)
        wob[:C] = Wo_b[k]
        selk = np.zeros((8, 128), np.float32)
        selk[k] = 1.0
        m = dict(shared)
        m.update({
            "wd_t": _bf16(wd),
            "wo_t": _bf16(wo.T.reshape(2, 128, CP)),
            "wob_t": wob.reshape(CP // 128, 128),
            "wg_t": _bf16(Wg_w[k].reshape(2, 128, 1)),
            "wgb_t": np.full((1, 1), Wg_b[k], np.float32),
            "db_t": dimred_b[k].reshape(1, N),
            "selk": selk,
        })
        in_maps.append(m)
    return in_maps


def assemble_outputs(results):
    hyp = np.stack(
        [r["out_hyp"].reshape(CP, B)[:C].T for r in results], axis=1
    )                                                   # [B, K, C]
    conf = np.stack([r["out_conf"][0] for r in results], axis=1)[..., None]
    loss = results[0]["out_loss"][0][:, None]           # [B, 1]
    return (
        np.ascontiguousarray(hyp, np.float32),
        np.ascontiguousarray(conf, np.float32),
        np.ascontiguousarray(loss, np.float32),
    )


_GRAPH_CACHE = {}


def get_graph():
    if "nc" not in _GRAPH_CACHE:
        nc = build_graph()
        nc.finalize()
        _GRAPH_CACHE["nc"] = nc
    return _GRAPH_CACHE["nc"]


def kernel(**inputs):
    nc = get_graph()
    in_maps = build_host_inputs(**inputs)
    res = run_bass_kernel_spmd(nc, in_maps, core_ids=list(range(NCORES)))
    return assemble_outputs(res.results)


# revision 25
# speedup vs baseline: 1.7302x; 1.2463x over previous
"""Trainium2 Bass kernel for nn_Attention_9844065042780.

Sharding: expert-parallel over the K=8 independent groups, one group per
NeuronCore (8 cores).  Each core receives the full activations x (reordered
host-side), the full conv_w (to compute the shared softmax attention maps
and the shared orthogonality loss), and only its own group's
dimred/Wo/Wg weights.  Outputs are disjoint per-core slices (hyp[:,k,:],
conf[:,k]) plus the (identical on every core) loss, gathered host-side.

Per-core math (k = this core's group):
  z[k', hw, b]   = sum_n conv_w[k', n] x[b, n, hw]            (PE)
  ah[k', hw, b]  = softmax_k'(z)                              (ACT exp, PE sum, DVE recip/mul)
  y[(hw,n), b]   = ah[k, hw, b] * x[b, n, hw]                 (PE row-broadcast + DVE mul)
  dim_red[b, m]  = sum_{hw,n} y * wd[m, n, hw] + db[m]        (PE, 417 accumulating matmuls)
  hyp[c, b]      = sum_n Wo[c, n] dim_red[b, n] + Wo_b[c]     (PE + ACT bias)
  conf[b]        = tanh(sum_n Wg[n] dim_red[b, n] + Wg_b)     (PE + ACT)
  loss[b]        = ||A_b^T A_b||_F^2 - sum((H^T A_b)^2),  A_b = ah[:, :, :, b]  (PE/ACT/DVE)

Layouts (host-prepared, hw padded 196->208 so slices are uniform 16*32=512):
  x_t  [2, 208, 128, 32]  bf16   x_t[nh, hw, nl, b] = x[b, nh*128+nl, hw]
  wd_t [2, 208, 128, 256] bf16   wd_t[nh, hw, nl, m] = dimred_w[k, m, nh*128+nl, hw]
"""

import os
import numpy as np
import ml_dtypes
from contextlib import ExitStack

from concourse import bass, bacc, tile, mybir
from concourse.bass_utils import run_bass_kernel_spmd

F32 = mybir.dt.float32
BF16 = mybir.dt.bfloat16
AF = mybir.ActivationFunctionType
ALU = mybir.AluOpType
AX = mybir.AxisListType

B, N, H, W, K, C = 32, 256, 14, 14, 8, 1000
HW, HWP, CP = 196, 196, 1024
SW = 14                      # hw positions per slice (one h row)
NSL = HWP // SW              # 14 slices
SLW = SW * B                 # 448 free columns per slice
NCORES = 8


def build_graph():
    nc = bacc.Bacc("TRN2", target_bir_lowering=False, debug=False)

    def inp(name, shape, dtype):
        return nc.dram_tensor(name, shape, dtype, kind="ExternalInput").ap()

    def outp(name, shape, dtype):
        return nc.dram_tensor(name, shape, dtype, kind="ExternalOutput").ap()

    x_d = inp("x_t", [2, 128, HWP, B], BF16)
    wd_d = inp("wd_t", [2, 128, HWP, N], BF16)
    cw_d = inp("cw_t", [2, 128, K], BF16)
    wo_d = inp("wo_t", [2, 128, CP], F32)
    wob_d = inp("wob_t", [CP // 128, 128], F32)
    wg_d = inp("wg_t", [2, 128, 1], F32)
    wgb_d = inp("wgb_t", [1, 1], F32)
    db_d = inp("db_t", [1, N], F32)
    ones8_d = inp("ones8", [8, 8], F32)
    selk_d = inp("selk", [8, 128], BF16)
    hsel_d = inp("hsel", [K * H, K], F32)
    ones14_d = inp("ones14", [14, 1], F32)
    mones8_d = inp("mones8", [8, 1], F32)
    onesb_d = inp("onesb", [1, B], F32)
    id32_d = inp("id32", [32, 32], F32)

    hyp_d = outp("out_hyp", [CP // 128, 128, B], F32)
    conf_d = outp("out_conf", [1, B], F32)
    loss_d = outp("out_loss", [1, B], F32)

    with tile.TileContext(nc) as tc, ExitStack() as ctx:
        const = ctx.enter_context(tc.tile_pool(name="const", bufs=1))
        persist = ctx.enter_context(tc.tile_pool(name="persist", bufs=1))
        dram = ctx.enter_context(tc.tile_pool(name="dram", bufs=1, space="DRAM"))
        dr_pool = ctx.enter_context(
            tc.tile_pool(name="dr_psum", bufs=1, space="PSUM")
        )

        # ---- constants into SBUF ----
        cw_sb = const.tile([128, 2 * K], BF16)
        nc.sync.dma_start(cw_sb.rearrange("p (t c) -> p t c", t=2),
                          cw_d.rearrange("t p c -> p t c"))
        wo_sb = const.tile([128, 2 * CP], F32)
        nc.sync.dma_start(wo_sb.rearrange("p (t c) -> p t c", t=2),
                          wo_d.rearrange("t p c -> p t c"))
        wob_sb = const.tile([128, CP // 128], F32)
        nc.sync.dma_start(wob_sb, wob_d.rearrange("c p -> p c"))
        wg_sb = const.tile([128, 2], F32)
        nc.sync.dma_start(wg_sb.rearrange("p (t one) -> p t one", t=2),
                          wg_d.rearrange("t p one -> p t one"))
        wgb_sb = const.tile([1, 1], F32)
        nc.sync.dma_start(wgb_sb, wgb_d)
        db_sb = const.tile([1, N], F32)
        nc.sync.dma_start(db_sb, db_d)
        ones8_sb = const.tile([8, 8], F32)
        nc.sync.dma_start(ones8_sb, ones8_d)
        selk_sb = const.tile([8, 128], BF16)
        nc.sync.dma_start(selk_sb, selk_d)
        hsel_sb = const.tile([K * H, K], F32)
        nc.sync.dma_start(hsel_sb, hsel_d)
        ones14_sb = const.tile([14, 1], F32)
        nc.sync.dma_start(ones14_sb, ones14_d)
        mones8_sb = const.tile([8, 1], F32)
        nc.sync.dma_start(mones8_sb, mones8_d)
        onesb_sb = const.tile([1, B], F32)
        nc.sync.dma_start(onesb_sb, onesb_d)
        id32_sb = const.tile([32, 32], F32)
        nc.sync.dma_start(id32_sb, id32_d)

        # full attention map, all K groups: [8, (hw, b)] fp32
        ah_full = persist.tile([K, HWP * B], F32)

        # x resident in SBUF: [128, (u, hw, b)], loaded in 2 line-rate DMAs
        x_all = persist.tile([128, 2 * HWP * B], BF16)
        for u in range(2):
            nc.sync.dma_start(
                x_all[:, u * HWP * B:(u + 1) * HWP * B], x_d[u])

        # dim_red accumulator [b, m]
        dimred_ps = dr_pool.tile([B, N], F32, space="PSUM")
        # bias seeding matmul: dimred[b, m] = 1 * db[m]
        nc.tensor.matmul(
            dimred_ps, lhsT=onesb_sb, rhs=db_sb, start=True, stop=False,
            skip_group_check=True,
        )

        F32R = mybir.dt.float32r
        with ExitStack() as mctx:
            wdp = mctx.enter_context(tc.tile_pool(name="wdp", bufs=8))
            ep = mctx.enter_context(tc.tile_pool(name="ep", bufs=3))
            abf = mctx.enter_context(tc.tile_pool(name="abf", bufs=3))
            rp = mctx.enter_context(tc.tile_pool(name="rp", bufs=3))
            yp = mctx.enter_context(tc.tile_pool(name="yp", bufs=4))
            zp = mctx.enter_context(tc.tile_pool(name="zp", bufs=2, space="PSUM"))
            sp = mctx.enter_context(tc.tile_pool(name="sp", bufs=2, space="PSUM"))
            abp = mctx.enter_context(tc.tile_pool(name="abp", bufs=2, space="PSUM"))

            for s in range(NSL):
                hw0 = s * SW

                def xw(u):
                    return x_all[:, u * HWP * B + hw0 * B:
                                 u * HWP * B + (hw0 + SW) * B]

                # logits z[k', (hw b)]
                z_ps = zp.tile([K, SLW], F32, tag="z", space="PSUM")
                nc.tensor.matmul(
                    z_ps, lhsT=cw_sb[:, 0:K], rhs=xw(0),
                    start=True, stop=False,
                )
                nc.tensor.matmul(
                    z_ps, lhsT=cw_sb[:, K:2 * K], rhs=xw(1),
                    start=False, stop=True,
                )
                e_sb = ep.tile([K, SLW], F32, tag="e")
                nc.scalar.activation(e_sb, z_ps, AF.Exp)
                ssum_ps = sp.tile([K, SLW], F32, tag="ss", space="PSUM")
                nc.tensor.matmul(ssum_ps, lhsT=ones8_sb, rhs=e_sb,
                                 start=True, stop=True)
                r_sb = rp.tile([K, SLW], F32, tag="r")
                nc.vector.reciprocal_approx_fast(r_sb, ssum_ps)
                ahw = ah_full[:, s * SLW:(s + 1) * SLW]
                nc.vector.tensor_tensor(out=ahw, in0=e_sb, in1=r_sb, op=ALU.mult)
                # bf16 copy (on ACT) so the broadcast matmul runs at 1 cyc/row
                ah_bf = abf.tile([K, SLW], BF16, tag="ahbf")
                nc.scalar.copy(ah_bf, ahw)
                # broadcast row k to 128 partitions
                ahb_ps = abp.tile([128, SLW], F32, tag="ab", space="PSUM")
                nc.tensor.matmul(ahb_ps, lhsT=selk_sb, rhs=ah_bf,
                                 start=True, stop=True)
                for nh in range(2):
                    y_sb = yp.tile([128, SLW], BF16, tag="y")
                    nc.vector.tensor_tensor(
                        out=y_sb, in0=xw(nh), in1=ahb_ps, op=ALU.mult,
                    )
                    wd_sb = wdp.tile([128, SW * N], BF16, tag="wd")
                    nc.sync.dma_start(wd_sb, wd_d[nh, :, hw0:hw0 + SW, :])
                    last_slice = s == NSL - 1
                    for j in range(SW):
                        nc.tensor.matmul(
                            dimred_ps,
                            lhsT=y_sb[:, j * B:(j + 1) * B],
                            rhs=wd_sb[:, j * N:(j + 1) * N],
                            start=False,
                            stop=(last_slice and nh == 1 and j == SW - 1),
                            skip_group_check=True,
                        )

        # ---------------- tail ----------------
        with ExitStack() as tctx:
            tp = tctx.enter_context(tc.tile_pool(name="tail_sb", bufs=1))
            tpp = tctx.enter_context(
                tc.tile_pool(name="tail_ps", bufs=1, space="PSUM")
            )
            hp = tctx.enter_context(
                tc.tile_pool(name="hyp_ps", bufs=2, space="PSUM")
            )
            hs = tctx.enter_context(tc.tile_pool(name="hyp_sb", bufs=2))

            # dim_red -> SBUF (f32), then transpose to [n, b]
            dr_sb = tp.tile([B, N], F32)
            nc.vector.tensor_copy(dr_sb, dimred_ps)
            drT_sb = tp.tile([128, 2 * B], F32)
            for nh in range(2):
                drT_ps = tpp.tile([128, B], F32, tag="drT", space="PSUM")
                nc.tensor.transpose(
                    drT_ps, dr_sb[:, nh * 128:(nh + 1) * 128], id32_sb
                )
                nc.vector.tensor_copy(drT_sb[:, nh * B:(nh + 1) * B], drT_ps)

            # hyp
            for c in range(CP // 128):
                hyp_ps = hp.tile([128, B], F32, tag="hyp", space="PSUM")
                nc.tensor.matmul(
                    hyp_ps, lhsT=wo_sb[:, c * 128:(c + 1) * 128],
                    rhs=drT_sb[:, 0:B], start=True, stop=False,
                )
                nc.tensor.matmul(
                    hyp_ps, lhsT=wo_sb[:, CP + c * 128:CP + (c + 1) * 128],
                    rhs=drT_sb[:, B:2 * B], start=False, stop=True,
                )
                hyp_sb = hs.tile([128, B], F32, tag="hyps")
                nc.scalar.activation(hyp_sb, hyp_ps, AF.Identity,
                                     bias=wob_sb[:, c:c + 1])
                nc.sync.dma_start(hyp_d[c], hyp_sb)

            # conf
            conf_ps = tpp.tile([1, B], F32, tag="conf", space="PSUM")
            nc.tensor.matmul(conf_ps, lhsT=wg_sb[:, 0:1], rhs=drT_sb[:, 0:B],
                             start=True, stop=False)
            nc.tensor.matmul(conf_ps, lhsT=wg_sb[:, 1:2], rhs=drT_sb[:, B:2 * B],
                             start=False, stop=True)
            conf_sb = tp.tile([1, B], F32)
            nc.scalar.activation(conf_sb, conf_ps, AF.Tanh, bias=wgb_sb[:, 0:1])
            nc.sync.dma_start(conf_d, conf_sb)

            # ---- loss ----
            # regather ah (hw < 196 region) to [(k h), (w b)] via DRAM bounce
            ah_dram = dram.tile([K, H * W * B], F32, space="DRAM")
            nc.sync.dma_start(ah_dram, ah_full[:, 0:H * W * B])
            A2 = tp.tile([K * H, W * B], F32)
            nc.sync.dma_start(
                A2, ah_dram.rearrange("k (h rest) -> (k h) rest", h=H)
            )
            A2v = A2.rearrange("p (w b) -> p w b", b=B)
            G_ps = tpp.tile([W, B * W], F32, tag="G", space="PSUM")
            for b in range(B):
                ab = A2v[:, :, b:b + 1]
                nc.tensor.matmul(G_ps[:, b * W:(b + 1) * W], lhsT=ab, rhs=ab,
                                 start=True, stop=True, skip_group_check=True)
            S_ps = tpp.tile([K, W * B], F32, tag="S", space="PSUM")
            nc.tensor.matmul(S_ps, lhsT=hsel_sb, rhs=A2, start=True, stop=True)
            Gsq = tp.tile([W, B * W], F32)
            nc.scalar.activation(Gsq, G_ps, AF.Square)
            Ssq = tp.tile([K, W * B], F32)
            nc.scalar.activation(Ssq, S_ps, AF.Square)
            Gred = tp.tile([W, B], F32)
            nc.vector.tensor_reduce(
                Gred, Gsq.rearrange("p (b v) -> p b v", b=B),
                axis=AX.X, op=ALU.add,
            )
            Sred = tp.tile([K, B], F32)
            nc.vector.tensor_reduce(
                Sred, Ssq.rearrange("p (w b) -> p b w", b=B),
                axis=AX.X, op=ALU.add,
            )
            l_ps = tpp.tile([1, B], F32, tag="l", space="PSUM")
            nc.tensor.matmul(l_ps, lhsT=ones14_sb, rhs=Gred,
                             start=True, stop=False)
            nc.tensor.matmul(l_ps, lhsT=mones8_sb, rhs=Sred,
                             start=False, stop=True)
            loss_sb = tp.tile([1, B], F32)
            nc.vector.tensor_copy(loss_sb, l_ps)
            nc.sync.dma_start(loss_d, loss_sb)

    return nc


def _bf16(a):
    return np.ascontiguousarray(a.astype(ml_dtypes.bfloat16))


def build_host_inputs(x, conv_w, dimred_w, dimred_b, Wo_w, Wo_b, Wg_w, Wg_b):
    """Returns in_maps: one dict per core."""
    x = np.asarray(x, np.float32)
    conv_w = np.asarray(conv_w, np.float32)
    dimred_w = np.asarray(dimred_w, np.float32)
    dimred_b = np.asarray(dimred_b, np.float32)
    Wo_w = np.asarray(Wo_w, np.float32)
    Wo_b = np.asarray(Wo_b, np.float32)
    Wg_w = np.asarray(Wg_w, np.float32)
    Wg_b = np.asarray(Wg_b, np.float32)

    # x_t[nh, nl, hw, b] = x[b, nh*128+nl, hw]  (partition-major contiguous)
    xt = x.transpose(1, 2, 3, 0).reshape(N, HW, B)          # [n, hw, b]
    xt = _bf16(xt.reshape(2, 128, HWP, B))

    # conv_w^T [2, 128, K]
    cwt = _bf16(conv_w.T.reshape(2, 128, K))

    shared = {
        "x_t": xt,
        "cw_t": cwt,
        "ones8": np.ones((8, 8), np.float32),
        "hsel": np.repeat(np.eye(K, dtype=np.float32), H, axis=0),
        "ones14": np.ones((14, 1), np.float32),
        "mones8": -np.ones((8, 1), np.float32),
        "onesb": np.ones((1, B), np.float32),
        "id32": np.eye(32, dtype=np.float32),
    }

    in_maps = []
    for k in range(NCORES):
        # wd_t[nh, nl, hw, m] = dimred_w[k, m, n, hw]  (partition-major contiguous)
        wd = dimred_w[k].reshape(N, N, HW).transpose(1, 2, 0)   # [n, hw, m]
        wd = wd.reshape(2, 128, HWP, N)
        wo = np.zeros((CP, N), np.float32)
        wo[:C] = Wo_w[k]
        wob = np.zeros((CP,), np.float32)
        wob[:C] = Wo_b[k]
        selk = np.zeros((8, 128), np.float32)
        selk[k] = 1.0
        m = dict(shared)
        m.update({
            "wd_t": _bf16(wd),
            "wo_t": np.ascontiguousarray(wo.T.reshape(2, 128, CP)),
            "wob_t": wob.reshape(CP // 128, 128),
            "wg_t": np.ascontiguousarray(Wg_w[k].reshape(2, 128, 1)),
            "wgb_t": np.full((1, 1), Wg_b[k], np.float32),
            "db_t": dimred_b[k].reshape(1, N),
            "selk": _bf16(selk),
        })
        in_maps.append(m)
    return in_maps


def assemble_outputs(results):
    hyp = np.stack(
        [r["out_hyp"].reshape(CP, B)[:C].T for r in results], axis=1
    )                                                   # [B, K, C]
    conf = np.stack([r["out_conf"][0] for r in results], axis=1)[..., None]
    loss = results[0]["out_loss"][0][:, None]           # [B, 1]
    return (
        np.ascontiguousarray(hyp, np.float32),
        np.ascontiguousarray(conf, np.float32),
        np.ascontiguousarray(loss, np.float32),
    )


_GRAPH_CACHE = {}


def get_graph():
    if "nc" not in _GRAPH_CACHE:
        nc = build_graph()
        nc.finalize()
        _GRAPH_CACHE["nc"] = nc
    return _GRAPH_CACHE["nc"]


def kernel(**inputs):
    nc = get_graph()
    in_maps = build_host_inputs(**inputs)
    res = run_bass_kernel_spmd(nc, in_maps, core_ids=list(range(NCORES)))
    return assemble_outputs(res.results)
